# revision 5
# baseline (speedup 1.0000x reference)
"""Row-wise cosine similarity on 8 TRN2 NeuronCores.

out[n] = sum_d(p[n,d]*h[n,d]) / (max(||p[n]||,eps) * max(||h[n]||,eps))
with N=65536, D=1024, eps=1e-12 (torch F.normalize semantics).

Sharding: rows split evenly across 8 cores (data parallel, no comms).

Port-15 dodge: HW traces show SDMA engine 15 sustains only ~21.8 B/ns
vs ~26.5 for engines 0-14 (periodic half-rate bursts), and with a
uniform 128-partition layout it gates the whole input stream (~349
GB/s ceiling). Engine k serves SBUF AXI port k; port 15 serves
partitions {92-95, 124-127}. This kernel therefore streams inputs
onto 120 partitions only ({0..91, 96..123}), giving 15 engines at
line rate (~398 GB/s ceiling, before any HBM cap).

Row layout (contiguous rows per partition, so the per-row results DMA
out contiguously): three affine runs
  run1: partitions  0..31,  69 rows each, base 69p      (rows 0..2207)
  run2: partitions 32..91,  68 rows each, base 2208+68(p-32)
  run3: partitions 96..123, 68 rows each, base 6288+68(p-96)
Tile column t (0..68) holds row base+t; column 68 exists only on
run1's 32 partitions and is processed FIRST (group 0) so the stream
ends with full-width tapered groups and a minimal compute drain.

Raw bass (no Tile scheduler): walrus codegen accepts at most ONE sync
wait per instruction. Partial-partition DMAs have unknown then_inc
fan-out (the usual +16 is one inc per SDMA engine), so no data DMA
carries a semaphore. Instead each group's p-runs and h-runs are
followed by a full-128-partition 4B/partition SBUF->SBUF "fence" DMA
that increments the group sem by 16. The sync HWDGE queue is FIFO per
SDMA engine, so a fence completing on every engine implies all
descriptors queued before it have drained: one sem, one wait, covers
the whole group.

Engine balance (HW-measured: ACT square+accum 1.41us, DVE
scalar_tensor_tensor+accum 1.28us per [*,1024] f32 tile): ACT does
||p||^2 for every column plus ~1/4 of the ||h||^2 columns; DVE does
p.h and the rest of ||h||^2. The tail columns alternate hh between
the engines so both finish together right after the last byte lands.
Epilogue: ph * rsqrt(pp*hh) with ACT sqrt + DVE reciprocal + one
Newton-Raphson step.
"""

import numpy as np

try:
    import concourse.bass as bass
except ImportError:  # fresh grading dir: toolchain lives in /opt
    import sys

    sys.path.insert(0, "/opt/trn_rl_repo")
    import concourse.bass as bass

from contextlib import ExitStack

from concourse import mybir
from concourse.bass_utils import run_bass_kernel_spmd

N, D = 65536, 1024
NCORES = 8
ROWS = N // NCORES  # 8192 rows per core
B = 5  # in-flight group buffers
GMAX = 4  # max tile-columns per group: [124, 2, 4, 1024] f32 = 32KB/partition
EPS2 = 1e-24  # eps^2; max(||x||,eps) == sqrt(max(||x||^2, eps^2)) here

# (p_start, p_end, rows_per_partition, row_base) — port 15 (partitions
# 92-95 and 124-127) gets zero input bytes.
RUNS = [(0, 32, 69, 0), (32, 92, 68, 2208), (96, 124, 68, 6288)]
NP_USED = 124  # compute ops span partitions 0..123 (92-95 hold garbage)
TCOLS = 69  # tile columns; column 68 valid only on partitions 0..31

_NC_CACHE = {}


def _groups():
    """Column schedule. The run1-only remainder column goes first (tiny
    group: early compute start); then cols 0..67 with a small front
    group and a 2/2/1/1 back taper so the post-stream drain is short."""
    gs = [[68], [0, 1]]
    c = 2
    while c + 4 <= 62:
        gs.append(list(range(c, c + 4)))
        c += 4
    gs += [[62, 63], [64, 65], [66], [67]]
    assert sorted(x for g in gs for x in g) == list(range(TCOLS))
    return gs


def _hh_on_act(t):
    """Which columns' ||h||^2 runs on ACT instead of DVE. One per full
    body group keeps DVE/ACT balanced under the DMA period; in the
    taper (61..66) alternate so both engines drain together; the last
    columns stay on DVE (same-engine pipelining into the epilogue)."""
    return (t % 4 == 3 and t < 61) or t in (61, 63, 65)


def _build_bass(detect_races=False):
    fp32 = mybir.dt.float32
    Sq = mybir.ActivationFunctionType.Square
    Sqrt = mybir.ActivationFunctionType.Sqrt
    mult = mybir.AluOpType.mult

    groups = _groups()
    NG = len(groups)

    nc = bass.Bass(detect_race_conditions=detect_races)
    prem = nc.declare_dram_parameter("premise", [ROWS, D], fp32, isOutput=False)
    hyp = nc.declare_dram_parameter("hypothesis", [ROWS, D], fp32, isOutput=False)
    outp = nc.declare_dram_parameter("out", [ROWS], fp32, isOutput=True)

    # per-run DRAM views: [n_parts, T, D] with rows contiguous per partition
    def run_views(tensor):
        views = []
        for p0, p1, T, base in RUNS:
            npart = p1 - p0
            v = tensor[base : base + npart * T, :].rearrange(
                "(p t) d -> p t d", p=npart
            )
            views.append(v)
        return views

    prem_v = run_views(prem)
    hyp_v = run_views(hyp)
    out_v = []
    for p0, p1, T, base in RUNS:
        npart = p1 - p0
        out_v.append(outp[base : base + npart * T].rearrange("(p t) -> p t", p=npart))

    with ExitStack() as mem:
        xs = [
            mem.enter_context(nc.sbuf_tensor(f"xs{i}", [NP_USED, 2, GMAX, D], fp32))
            for i in range(B)
        ]
        junk_a = mem.enter_context(nc.sbuf_tensor("junk_a", [NP_USED, D], fp32))
        junk_v = mem.enter_context(nc.sbuf_tensor("junk_v", [NP_USED, D], fp32))
        fence = mem.enter_context(nc.sbuf_tensor("fence", [128, 2], fp32))
        r_pp = mem.enter_context(nc.sbuf_tensor("r_pp", [NP_USED, TCOLS], fp32))
        r_hh = mem.enter_context(nc.sbuf_tensor("r_hh", [NP_USED, TCOLS], fp32))
        r_ph = mem.enter_context(nc.sbuf_tensor("r_ph", [NP_USED, TCOLS], fp32))
        d2 = mem.enter_context(nc.sbuf_tensor("d2", [NP_USED, TCOLS], fp32))
        sd = mem.enter_context(nc.sbuf_tensor("sd", [NP_USED, TCOLS], fp32))
        yv = mem.enter_context(nc.sbuf_tensor("yv", [NP_USED, TCOLS], fp32))
        t1 = mem.enter_context(nc.sbuf_tensor("t1", [NP_USED, TCOLS], fp32))
        res = mem.enter_context(nc.sbuf_tensor("res", [NP_USED, TCOLS], fp32))

        with ExitStack() as semctx:
            s_dma_p = [
                semctx.enter_context(nc.semaphore(f"s_dma_p{i}")) for i in range(8)
            ]
            s_dma_h = [
                semctx.enter_context(nc.semaphore(f"s_dma_h{i}")) for i in range(8)
            ]
            s_act = semctx.enter_context(nc.semaphore("s_act"))
            s_dve = semctx.enter_context(nc.semaphore("s_dve"))
            s_ch = semctx.enter_context(nc.semaphore("s_ch"))
            s_ep2 = semctx.enter_context(nc.semaphore("s_ep2"))
            s_res = semctx.enter_context(nc.semaphore("s_res"))
            s_out = semctx.enter_context(nc.semaphore("s_out"))
            # codegen requires sync info on every dynamic DMA; partial-
            # partition transfers have unknown inc fan-out, so data DMAs
            # dump their incs here and nobody waits on it.
            s_trash = semctx.enter_context(nc.semaphore("s_trash"))

            def issue_side(eng, g, side, views, sem):
                """DMA every run's slice of group g for one input, then a
                full-128-partition fence that carries the semaphore."""
                cols = groups[g]
                c0, ncol = cols[0], len(cols)
                assert cols == list(range(c0, c0 + ncol))
                for r, (p0, p1, T, base) in enumerate(RUNS):
                    if c0 >= T:
                        continue  # column 68: run1 only
                    hi = min(c0 + ncol, T)
                    eng.dma_start(
                        out=xs[g % B][p0:p1, side, : hi - c0, :],
                        in_=views[r][:, c0:hi, :],
                    ).then_inc(s_trash, 16)
                eng.dma_start(out=fence[:, 1:2], in_=fence[:, 0:1]).then_inc(sem, 16)

            with nc.Block() as block:

                @block.sync
                def _(eng: bass.BassEngine):
                    for g in range(NG):
                        if g >= B:
                            # DVE inc implies ACT done too (transitive)
                            eng.wait_ge(s_dve, g - B + 1)
                        issue_side(eng, g, 0, prem_v, s_dma_p[g % 8])
                        issue_side(eng, g, 1, hyp_v, s_dma_h[g % 8])
                    eng.wait_ge(s_res, 1)
                    for r, (p0, p1, T, base) in enumerate(RUNS):
                        eng.dma_start(out=out_v[r], in_=res[p0:p1, :T]).then_inc(
                            s_trash, 16
                        )
                    eng.dma_start(out=fence[:, 1:2], in_=fence[:, 0:1]).then_inc(
                        s_out, 16
                    )
                    eng.wait_ge(s_out, 16)

                @block.scalar
                def _(eng: bass.BassEngine):
                    for g in range(NG):
                        eng.wait_ge(s_dma_p[g % 8], 16 * (g // 8 + 1))
                        sl = xs[g % B]
                        cols = groups[g]
                        c0 = cols[0]
                        hh_mine = [t for t in cols if _hh_on_act(t)]
                        last = ("hh", hh_mine[-1]) if hh_mine else ("pp", cols[-1])
                        for t in cols:
                            ins = eng.activation(
                                out=junk_a[:, :],
                                in_=sl[:, 0, t - c0, :],
                                func=Sq,
                                accum_out=r_pp[:, t : t + 1],
                            )
                            if last == ("pp", t):
                                ins.then_inc(s_act, 1)
                        if hh_mine:
                            eng.wait_ge(s_dma_h[g % 8], 16 * (g // 8 + 1))
                            for t in hh_mine:
                                ins = eng.activation(
                                    out=junk_a[:, :],
                                    in_=sl[:, 1, t - c0, :],
                                    func=Sq,
                                    accum_out=r_hh[:, t : t + 1],
                                )
                                if last == ("hh", t):
                                    ins.then_inc(s_act, 1)
                    # epilogue: sqrt of pp*hh (after DVE built d2 = 2nd chain inc)
                    eng.wait_ge(s_ch, 2)
                    eng.activation(out=sd[:], in_=d2[:], func=Sqrt).then_inc(s_ep2, 1)

                @block.vector
                def _(eng: bass.BassEngine):
                    mx = mybir.AluOpType.max
                    add = mybir.AluOpType.add
                    for g in range(NG):
                        # h fence is issued after every run of both sides on
                        # the same FIFO queue: one wait covers the group.
                        eng.wait_ge(s_dma_h[g % 8], 16 * (g // 8 + 1))
                        sl = xs[g % B]
                        cols = groups[g]
                        c0 = cols[0]
                        ops = []  # (kind, t)
                        for t in cols:
                            ops.append(("ph", t))
                            if not _hh_on_act(t):
                                ops.append(("hh", t))
                        for k, (kind, t) in enumerate(ops):
                            if k == len(ops) - 1:
                                # DVE completion of group g implies ACT done
                                eng.wait_ge(s_act, g + 1)
                            side0 = 0 if kind == "ph" else 1
                            accum = r_ph if kind == "ph" else r_hh
                            ins = eng.scalar_tensor_tensor(
                                out=junk_v[:, :],
                                in0=sl[:, side0, t - c0, :],
                                scalar=1.0,
                                in1=sl[:, 1, t - c0, :],
                                op0=mult,
                                op1=mult,
                                accum_out=accum[:, t : t + 1],
                            )
                            if k == len(ops) - 1:
                                ins.then_inc(s_dve, 1)
                    # epilogue: res = ph * rsqrt(max(pp,e)*max(hh,e)).
                    # DVE pipelines same-engine dependent ops, so every
                    # same-engine RAW needs a sem; s_ch counts epilogue
                    # DVE completions.
                    eng.wait_ge(s_dve, NG)
                    eng.tensor_scalar_max(
                        out=r_hh[:], in0=r_hh[:], scalar1=EPS2
                    ).then_inc(s_ch, 1)
                    eng.wait_ge(s_ch, 1)
                    eng.scalar_tensor_tensor(
                        out=d2[:], in0=r_pp[:], scalar=EPS2, in1=r_hh[:],
                        op0=mx, op1=mult,
                    ).then_inc(s_ch, 1)
                    eng.wait_ge(s_ep2, 1)
                    eng.reciprocal(out=yv[:], in_=sd[:]).then_inc(s_ch, 1)
                    # Newton step for rsqrt: y *= 1.5 - 0.5*d2*y*y
                    eng.wait_ge(s_ch, 3)
                    eng.tensor_mul(t1[:], yv[:], yv[:]).then_inc(s_ch, 1)
                    eng.wait_ge(s_ch, 4)
                    eng.scalar_tensor_tensor(
                        out=t1[:], in0=d2[:], scalar=-0.5, in1=t1[:],
                        op0=mult, op1=mult,
                    ).then_inc(s_ch, 1)
                    eng.wait_ge(s_ch, 5)
                    eng.scalar_tensor_tensor(
                        out=yv[:], in0=t1[:], scalar=1.5, in1=yv[:],
                        op0=add, op1=mult,
                    ).then_inc(s_ch, 1)
                    eng.wait_ge(s_ch, 6)
                    eng.tensor_mul(res[:], r_ph[:], yv[:]).then_inc(s_res, 1)

    return nc


def _get_nc():
    if "nc" not in _NC_CACHE:
        _NC_CACHE["nc"] = _build_bass()
    return _NC_CACHE["nc"]


def _run(premise, hypothesis, trace=False, **kwargs):
    premise = np.ascontiguousarray(np.asarray(premise, dtype=np.float32))
    hypothesis = np.ascontiguousarray(np.asarray(hypothesis, dtype=np.float32))
    assert premise.shape == (N, D) and hypothesis.shape == (N, D)
    nc = _get_nc()
    in_maps = [
        {
            "premise": premise[c * ROWS : (c + 1) * ROWS],
            "hypothesis": hypothesis[c * ROWS : (c + 1) * ROWS],
        }
        for c in range(NCORES)
    ]
    r = run_bass_kernel_spmd(nc, in_maps, list(range(NCORES)), trace=trace, **kwargs)
    out = np.concatenate([r.results[c]["out"] for c in range(NCORES)])
    return out, r


def kernel(premise, hypothesis):
    out, _ = _run(premise, hypothesis)
    return out
